# revision 17
# baseline (speedup 1.0000x reference)
"""Trainium2 Bass kernel for nn_Cross_At_50208167690358 (cosine-sim cross attention).

Math (per reference): q = x@Wq+bq; k,v = y@Wkv+bkv (split); q,k l2-normalized
over head dim (8); attn = softmax((q_hat . k_hat) * temp); out = attn @ v.
B=2, HW=4096, C=64, H=8, hd=8.

Key trick: scores s = q_hat.k_hat are cosine similarities, |s| <= 1.
Replace exp(t*s) by a degree-3 polynomial p(s) (Chebyshev interpolant of
exp(t*s) on [-1,1], fit on host from the runtime temperature).  p(s) expands
exactly over a 165-dim monomial feature map phi (1 + 8 + 36 + 120 monomials
of q_hat/k_hat up to degree 3):

    sum_j p(s_ij) * v_j = phi(q_i) . ( diag(w) @ Phi_k^T @ V_aug )

so the whole attention collapses to *linear attention*: no 4096x4096 score
matrix and no exp stream (the baseline was ScalarE-bound on 33.5M exps/core).
Accuracy (validated vs reference on CPU, incl. bf16 rounding): ~2.4e-3 rel.

Sharding: 16 (b,h) units -> 2 per core (cores share batch b = core // 4).

Per-core pipeline (units fused, bf16 data, fp32 PSUM accumulation):
  A: project k/v/q in natural layout (xT/yT stationary, ones-row bias trick;
     v gets an extra all-ones column for the softmax denominator).
  B: l2-normalize via DVE square/reduce + ACT Ln/Exp(-0.5), write q_hat/k_hat
     into feature-major phi tensors [128, F, 64] (64 = unit*32 + i-tile).
  C: build monomial features on DVE: 16-17 broadcast-multiply instrs per side.
  D: k-side: Mt[9,165] (per unit) += V_aug_tile^T @ Phi_k_tile  (32 matmuls).
  E: Mt -> SBUF -> PE-transpose -> apply poly weights -> M_w [165,9] bf16.
  F: PE-transpose Phi_q tiles to PSUM ([f, i] layout).
  G: copy transposed Phi_q^T to SBUF (DVE chunk1 / ACT chunk2).
  H: out_nat[128,9] += Phi_q^T-chunk (stationary) @ M_w-chunk  (2 per i-tile).
  I: out = num/denom via DVE reciprocal+mul; DMA out.
"""

import sys

if "/opt/trn_rl_repo" not in sys.path:
    sys.path.insert(0, "/opt/trn_rl_repo")

from contextlib import ExitStack
from math import factorial

import numpy as np
import ml_dtypes

import concourse.bass as bass  # noqa: F401
from concourse import bacc, mybir
import concourse.tile as tile
from concourse.bass_utils import run_bass_kernel_spmd
from concourse.masks import make_identity

P = 128
HW = 4096
C = 64
H = 8
D = 8          # head dim
B = 2
NCORES = 8
NU = 2         # (b, h) units per core
NIT = HW // P  # 32 i-tiles
NCOL = NU * NIT  # 64 fused (unit, i-tile) columns

DEG = 3
F = 165        # 1 + 8 + 36 + 120 monomials up to degree 3
F1 = 128       # chunk 1 of the feature dim
F2 = F - F1    # 37

F32 = mybir.dt.float32
BF16 = mybir.dt.bfloat16
AF = mybir.ActivationFunctionType

# feature-block offsets (degree-2 / degree-3 prefix tables)
W2 = [8 - d for d in range(8)]                      # widths of deg-2 blocks
OFF2 = [9 + sum(W2[:d]) for d in range(8)]          # deg-2 block starts
W3 = [sum(W2[d:]) for d in range(8)]                # widths of deg-3 blocks
OFF3 = [45 + sum(W3[:d]) for d in range(8)]         # deg-3 block starts
assert OFF3[-1] + W3[-1] == F

_CACHE = {}


def _feat_weights(t):
    """Poly-kernel weights w_f so that sum_f w_f phi_f(q) phi_f(k) ~ exp(t*q.k)
    for unit q, k. Chebyshev interpolant of exp(t*s) on [-1,1], degree 3."""
    cheb = np.polynomial.chebyshev.chebinterpolate(
        lambda s: np.exp(t * s), DEG)
    c = np.polynomial.chebyshev.cheb2poly(cheb)

    def multinom(idx):
        counts = {}
        for d in idx:
            counts[d] = counts.get(d, 0) + 1
        r = factorial(len(idx))
        for v in counts.values():
            r //= factorial(v)
        return r

    w = np.empty(F, np.float64)
    w[0] = c[0]
    for d in range(8):
        w[1 + d] = c[1]
    i = 9
    for d1 in range(8):
        for d2 in range(d1, 8):
            w[i] = c[2] * multinom((d1, d2))
            i += 1
    for d1 in range(8):
        for d2 in range(d1, 8):
            for d3 in range(d2, 8):
                w[i] = c[3] * multinom((d1, d2, d3))
                i += 1
    assert i == F
    return w.astype(np.float32)


def _emit_features(nc, phiA, phiB, split):
    """Monomial build on DVE. phiA holds features [0, split), phiB the rest.
    Linear slots (normalized vectors) live at phiA[:, 1:9, :]."""

    def hat(d):
        return phiA[:, 1 + d:2 + d, :]

    # degree 2: block d = hat[d] * hat[d..8]   (all below `split`)
    for d in range(8):
        w = 8 - d
        nc.vector.tensor_mul(
            phiA[:, OFF2[d]:OFF2[d] + w, :],
            hat(d).to_broadcast((P, w, NCOL)),
            phiA[:, 1 + d:9, :])
    # degree 3: block d = hat[d] * deg2[OFF2[d]:45]
    for d in range(8):
        w = W3[d]
        src = phiA[:, OFF2[d]:45, :]
        if OFF3[d] + w <= split:
            nc.vector.tensor_mul(
                phiA[:, OFF3[d]:OFF3[d] + w, :],
                hat(d).to_broadcast((P, w, NCOL)), src)
        elif OFF3[d] >= split:
            nc.vector.tensor_mul(
                phiB[:, OFF3[d] - split:OFF3[d] - split + w, :],
                hat(d).to_broadcast((P, w, NCOL)), src)
        else:
            wa = split - OFF3[d]
            nc.vector.tensor_mul(
                phiA[:, OFF3[d]:split, :],
                hat(d).to_broadcast((P, wa, NCOL)), src[:, 0:wa, :])
            nc.vector.tensor_mul(
                phiB[:, 0:w - wa, :],
                hat(d).to_broadcast((P, w - wa, NCOL)), src[:, wa:, :])


def build_program(reps=1, taps=()):
    nc = bacc.Bacc("TRN2", target_bir_lowering=False, debug=False,
                   num_devices=NCORES)
    xT_d = nc.dram_tensor("xT", [C + 1, HW], BF16, kind="ExternalInput").ap()
    yT_d = nc.dram_tensor("yT", [C + 1, HW], BF16, kind="ExternalInput").ap()
    wq_d = nc.dram_tensor("wq", [C + 1, NU, D], BF16, kind="ExternalInput").ap()
    wkv_d = nc.dram_tensor("wkv", [C + 1, NU, 2 * D + 1], BF16,
                           kind="ExternalInput").ap()
    wv1_d = nc.dram_tensor("wvec1", [F1, NU], F32, kind="ExternalInput").ap()
    wv2_d = nc.dram_tensor("wvec2", [F2, NU], F32, kind="ExternalInput").ap()
    out_d = nc.dram_tensor("out", [NU, HW, D], F32, kind="ExternalOutput").ap()

    with tile.TileContext(nc) as tc, ExitStack() as ctx:
        pools = {
            "const": ctx.enter_context(tc.tile_pool(name="const", bufs=1)),
            "main": ctx.enter_context(tc.tile_pool(name="main", bufs=1)),
            "work": ctx.enter_context(tc.tile_pool(name="work", bufs=2)),
            # PSUM budget (8 banks): pk 1x2 + pv 2 + mt 1 + ring 1x2 + sm 1 = 8
            "pk": ctx.enter_context(
                tc.tile_pool(name="pk", bufs=2, space="PSUM")),
            "pv": ctx.enter_context(
                tc.tile_pool(name="pv", bufs=1, space="PSUM")),
            "mt": ctx.enter_context(
                tc.tile_pool(name="mt", bufs=1, space="PSUM")),
            "ring": ctx.enter_context(
                tc.tile_pool(name="ring", bufs=2, space="PSUM")),
            "sm": ctx.enter_context(
                tc.tile_pool(name="sm", bufs=1, space="PSUM")),
        }

        def emit_all():
            const, main, work = pools["const"], pools["main"], pools["work"]
            xT = const.tile([C + 1, HW], BF16, name="xT")
            yT = const.tile([C + 1, HW], BF16, name="yT")
            wq = const.tile([C + 1, NU, D], BF16, name="wq")
            wkv = const.tile([C + 1, NU, 2 * D + 1], BF16, name="wkv")
            wv1 = const.tile([F1, NU], F32, name="wv1")
            wv2 = const.tile([F2, NU], F32, name="wv2")
            identB = const.tile([P, P], BF16, name="identB")
            ident9 = const.tile([9, 9], F32, name="ident9")
            nc.sync.dma_start(yT[:], yT_d)
            nc.sync.dma_start(xT[:], xT_d)
            nc.sync.dma_start(wq[:], wq_d)
            nc.sync.dma_start(wkv[:], wkv_d)
            nc.sync.dma_start(wv1[:], wv1_d)
            nc.sync.dma_start(wv2[:], wv2_d)
            make_identity(nc, identB[:])
            make_identity(nc, ident9[:])

            phiK = main.tile([P, F, NCOL], BF16, name="phiK")
            phiQA = main.tile([P, F1, NCOL], BF16, name="phiQA")
            phiQB = main.tile([P, F2, NCOL], BF16, name="phiQB")
            vN = main.tile([P, NU, NIT, D + 1], BF16, name="vN")
            phiT1 = main.tile([F1, NU, HW], BF16, name="phiT1")
            phiT2 = main.tile([F2, NU, HW], BF16, name="phiT2")
            Mw1 = main.tile([F1, NU, D + 1], BF16, name="Mw1")
            Mw2 = main.tile([F2, NU, D + 1], BF16, name="Mw2")
            out_sb = main.tile([P, NU, NIT, D], F32, name="out_sb")

            nc.gpsimd.memset(phiK[:, 0, :], 1.0)
            nc.gpsimd.memset(phiQA[:, 0, :], 1.0)

            # ---- A: projections (k first: feeds PE k-side earliest) ----
            def project(src, w_ap, ncols, tag, pad=None):
                # pad: per-i-tile column stride; must divide the 2KB PSUM
                # bank so no matmul output straddles a bank boundary.
                pad = pad or ncols
                ps = pools[tag].tile([P, NIT, pad], F32, tag=tag,
                                     name=f"ps{tag}")
                for it in range(NIT):
                    nc.tensor.matmul(
                        ps[:, it, 0:ncols], src[:, it * P:(it + 1) * P], w_ap,
                        start=True, stop=True)
                return ps

            def normalize(psv, phi_slots):
                # psv: [P, NIT, NU, 8] projection view (PSUM fp32)
                sq = work.tile([P, NIT, NU, D], F32, tag="sq")
                nc.scalar.activation(sq[:], psv, AF.Square)
                ssum = work.tile([P, NIT, NU], F32, tag="ssum")
                nc.vector.tensor_reduce(ssum[:], sq[:], mybir.AxisListType.X,
                                        mybir.AluOpType.add)
                lns = work.tile([P, NIT, NU], F32, tag="lns")
                nc.scalar.activation(lns[:], ssum[:], AF.Ln)
                inv = work.tile([P, NIT, NU], F32, tag="inv")
                nc.scalar.activation(inv[:], lns[:], AF.Exp, scale=-0.5)
                nc.vector.tensor_mul(
                    phi_slots, psv,
                    inv[:, :, :, None].to_broadcast((P, NIT, NU, D)))

            ps_k = project(yT, wkv[:, :, 0:D], NU * D, "pk")
            normalize(
                ps_k[:].rearrange("p it (u d) -> p it u d", u=NU),
                phiK[:, 1:9, :].rearrange("p d (u it) -> p it u d", u=NU))

            ps_v = project(yT, wkv[:, :, D:2 * D + 1], NU * (D + 1), "pv",
                           pad=32)
            nc.vector.tensor_copy(
                vN[:],
                ps_v[:, :, 0:NU * (D + 1)].rearrange(
                    "p it (u c) -> p u it c", u=NU))

            ps_q = project(xT, wq[:], NU * D, "pk")
            normalize(
                ps_q[:].rearrange("p it (u d) -> p it u d", u=NU),
                phiQA[:, 1:9, :].rearrange("p d (u it) -> p it u d", u=NU))

            # ---- C: monomial features ----
            import os as _os
            _ab = _os.environ.get("ABLATE", "")
            if "feat" not in _ab:
                _emit_features(nc, phiK, None, F)
                _emit_features(nc, phiQA, phiQB, F1)

            # ---- D: k-side Mt[9, F] per unit ----
            mt = pools["mt"].tile([D + 1, NU, F], F32, tag="mt")
            for u in range(NU):
                for it in range(NIT):
                    nc.tensor.matmul(
                        mt[:, u, :], vN[:, u, it, :],
                        phiK[:, :, u * NIT + it],
                        start=(it == 0), stop=(it == NIT - 1))

            # ---- E: Mt -> M_w (transpose + poly weights) ----
            mt_sb = work.tile([D + 1, NU, F], F32, tag="mtsb")
            nc.scalar.activation(mt_sb[:].rearrange("p a b -> p (a b)"),
                                 mt[:].rearrange("p a b -> p (a b)"), AF.Copy)
            mwtr = pools["sm"].tile([P, 2, NU, D + 1], F32, tag="mwtr")
            for u in range(NU):
                nc.tensor.transpose(mwtr[:, 0, u, :], mt_sb[:, u, 0:F1],
                                    ident9)
                nc.tensor.transpose(mwtr[0:F2, 1, u, :], mt_sb[:, u, F1:F],
                                    ident9)
            nc.vector.tensor_mul(
                Mw1[:], mwtr[:, 0, :, :],
                wv1[:, :, None].to_broadcast((F1, NU, D + 1)))
            nc.vector.tensor_mul(
                Mw2[:], mwtr[0:F2, 1, :, :],
                wv2[:, :, None].to_broadcast((F2, NU, D + 1)))

            # ---- F/G: transpose phi_q to [f, i] layout ----
            for u in range(NU if "fg" not in _ab else 0):
                for g in range(4):   # chunk1, groups of 8 i-tiles
                    tr = pools["ring"].tile([P, 8, P], BF16, tag="ring",
                                            name="tr1")
                    for s in range(8):
                        it = 8 * g + s
                        nc.tensor.transpose(
                            tr[:, s, :], phiQA[:, :, u * NIT + it], identB)
                    nc.vector.tensor_copy(
                        phiT1[:, u, g * 8 * P:(g + 1) * 8 * P], tr[:])
                for g in range(4):   # chunk2
                    tr = pools["ring"].tile([P, 8, P], BF16, tag="ring",
                                            name="tr2")
                    for s in range(8):
                        it = 8 * g + s
                        nc.tensor.transpose(
                            tr[0:F2, s, :], phiQB[:, :, u * NIT + it], identB)
                    nc.scalar.activation(
                        phiT2[:, u, g * 8 * P:(g + 1) * 8 * P],
                        tr[0:F2, :, :], AF.Copy)

            # ---- H/I: q-side matmuls + divide ----
            for u in range(NU if "hi" not in _ab else 0):
                for g in range(4):
                    onat = pools["ring"].tile([P, 8, D + 1], F32, tag="ring",
                                              name="onat")
                    for s in range(8):
                        it = 8 * g + s
                        nc.tensor.matmul(
                            onat[:, s, :],
                            phiT1[:, u, it * P:(it + 1) * P], Mw1[:, u, :],
                            start=True, stop=False)
                        nc.tensor.matmul(
                            onat[:, s, :],
                            phiT2[:, u, it * P:(it + 1) * P], Mw2[:, u, :],
                            start=False, stop=True)
                    rec = work.tile([P, 8, 1], F32, tag="rec")
                    nc.vector.reciprocal(rec[:], onat[:, :, D:D + 1])
                    nc.vector.tensor_mul(
                        out_sb[:, u, g * 8:(g + 1) * 8, :], onat[:, :, 0:D],
                        rec[:].to_broadcast((P, 8, D)))

            for u in range(NU):
                nc.sync.dma_start(
                    out_d[u].rearrange("(it ii) d -> ii it d", ii=P),
                    out_sb[:, u])

            tap_tiles = {"phiK": phiK, "phiQA": phiQA, "phiQB": phiQB,
                         "vN": vN, "mt_sb": mt_sb, "Mw1": Mw1, "Mw2": Mw2,
                         "phiT1": phiT1, "phiT2": phiT2}
            for tname in taps:
                tl = tap_tiles[tname]
                td = nc.dram_tensor(f"tap_{tname}", list(tl[:].shape),
                                    tl[:].dtype, kind="ExternalOutput").ap()
                nc.sync.dma_start(td, tl[:])

        if reps == 1:
            emit_all()
        else:
            with tc.For_i(0, reps, 1):
                emit_all()

    nc.compile()
    return nc


def _prep_inputs(x, y, Wq, bq, Wkv, bkv, temperature):
    """Host-side sharding/relayout + per-head poly-weight fit."""
    x = np.asarray(x, np.float32)
    y = np.asarray(y, np.float32)
    Wq = np.asarray(Wq, np.float32)
    bq = np.asarray(bq, np.float32)
    Wkv = np.asarray(Wkv, np.float32)
    bkv = np.asarray(bkv, np.float32)
    temps = np.asarray(temperature, np.float32).reshape(H)
    ones = np.ones((1, HW), dtype=np.float32)
    bf = ml_dtypes.bfloat16
    in_maps = []
    for c in range(NCORES):
        b = c // 4
        heads = [2 * (c % 4), 2 * (c % 4) + 1]
        xT = np.concatenate([np.ascontiguousarray(x[b].T), ones], 0)
        yT = np.concatenate([np.ascontiguousarray(y[b].T), ones], 0)
        wq = np.empty((C + 1, NU, D), np.float32)
        wkv = np.zeros((C + 1, NU, 2 * D + 1), np.float32)
        wvec = np.empty((F, NU), np.float32)
        for u, h in enumerate(heads):
            wq[:C, u, :] = Wq[:, D * h:D * (h + 1)]
            wq[C, u, :] = bq[D * h:D * (h + 1)]
            wkv[:C, u, 0:D] = Wkv[:, D * h:D * (h + 1)]
            wkv[C, u, 0:D] = bkv[D * h:D * (h + 1)]
            wkv[:C, u, D:2 * D] = Wkv[:, C + D * h:C + D * (h + 1)]
            wkv[C, u, D:2 * D] = bkv[C + D * h:C + D * (h + 1)]
            wkv[C, u, 2 * D] = 1.0     # ones column for the denominator
            wvec[:, u] = _feat_weights(float(temps[h]))
        in_maps.append({
            "xT": xT.astype(bf), "yT": yT.astype(bf),
            "wq": wq.astype(bf), "wkv": wkv.astype(bf),
            "wvec1": wvec[:F1], "wvec2": wvec[F1:],
        })
    return in_maps


def run(x, y, Wq, bq, Wkv, bkv, temperature, trace=False):
    if "nc" not in _CACHE:
        _CACHE["nc"] = build_program()
    nc = _CACHE["nc"]
    in_maps = _prep_inputs(x, y, Wq, bq, Wkv, bkv, temperature)
    res = run_bass_kernel_spmd(nc, in_maps, core_ids=list(range(NCORES)),
                               trace=trace)
    out = np.empty((B, HW, C), dtype=np.float32)
    for c in range(NCORES):
        b = c // 4
        heads = [2 * (c % 4), 2 * (c % 4) + 1]
        core_out = res.results[c]["out"]
        for u, h in enumerate(heads):
            out[b, :, D * h:D * (h + 1)] = core_out[u]
    return out, res


def kernel(x, y, Wq, bq, Wkv, bkv, temperature):
    out, _ = run(np.asarray(x), np.asarray(y), np.asarray(Wq), np.asarray(bq),
                 np.asarray(Wkv), np.asarray(bkv), np.asarray(temperature))
    return out
